# revision 1
# baseline (speedup 1.0000x reference)
"""Trainium2 Bass kernel for per-sample covariance pooling + fc + L2 norm.

Reference computation (per sample b of B=32):
    xc  = x[b] - mean(x[b], axis=0)            # x[b]: [N=20000, D=64]
    cov = xc.T @ xc / (N-1)                    # [64, 64]
    out = normalize(cov.flatten() @ W + b)     # [256]

Kernel formulation (scale/norm invariant):
    G = x.T @ x, s = sum(x, axis=0)            # one PE pass over x
    cov = (G - s s^T / N) / (N-1)
    out = normalize(cov.flatten() @ W + b)

Sharding: data-parallel over batch, 4 samples per core on 8 cores; W
and bias replicated. x is host-packed to fp8 e4m3 (end-to-end rel err
~2.3e-3 vs the 2e-2 gate). Two samples ride side by side per
partition row: chunk layout [x_a(64) | x_b(64) | ones(1)], so the
Gram matmul has a 128-column stationary operand -- exactly the shape
that triggers the compiler's Fast Weight Load (4 fp8/cycle; the
dominant LDWEIGHTS cost of tall-skinny Grams drops 4x) -- and one
matmul per 128 rows yields both samples' G blocks plus both column
sums (from the ones column) in a [128, 129] psum. DoubleRow mode is
deliberately NOT used: at free-dim 64 it disables FWL and measures
~3x slower (120 vs 40 ns/matmul).

All x DMAs are issued before the W DMAs on both HWDGE rings, so the
fc matmuls (which need all four samples' feats) hide under the W
stream instead of trailing the x stream.
"""

import sys

import numpy as np
import ml_dtypes

for _p in ("/opt/trn_rl_repo",):
    if _p not in sys.path:
        sys.path.append(_p)

# Problem shapes (hardcoded per contract).
B, N, D, OUT = 32, 20000, 64, 256
NCORES = 8
BPC = B // NCORES            # samples per core
NPAIR = BPC // 2             # sample pairs per core
P = 128                      # SBUF partitions / matmul contraction tile
NCH = (N + P - 1) // P       # 157 contraction chunks of 128 rows
NPAD = NCH * P               # 20096 rows after zero padding
FB = 2 * D + 1               # bytes per partition per chunk (pair + ones)
KC = (D * D) // P            # 32 fc contraction chunks
WSLICES = 8                  # W DMA slices (each covers 4 fc chunks)
# x DMA schedule per sample pair: (chunk offset, chunks). Last tile is
# small so the final G chunks finish right after the stream ends.
DMA_TILES = [(0, 28), (28, 28), (56, 28), (84, 28), (112, 28), (140, 17)]
FILL_PER_TILE = 1            # HAM-warming dummy matmuls per x tile

_CACHE = {}


def _split_drain_and_barrier(self, tick_clock, wait_clock):
    """Replacement for TileContext._drain_and_barrier emitting one drain per
    sem wait: this walrus vintage rejects >1 sync-wait per instruction."""
    import bass_rust
    import concourse.mybir as mybir

    drain_bi = self.nc.sync.drain()
    inst = drain_bi.ins
    wait_clock.add_sem_waits(
        drain_bi.ins, bass_rust.ScopedClock({None: tick_clock.global_clock})
    )
    waits = list(inst.sync_info.on_wait) if inst.sync_info else []
    if len(waits) > 1:
        # one pure sem-wait NoOp per extra wait (cheaper than extra drains)
        inst.sync_info = mybir.SyncInfo(on_wait=waits[:1], on_update=[])
        for w in waits[1:]:
            nop = mybir.InstNoOp(
                name=f"tailwait-{w.ant_name}",
                engine=mybir.EngineType.SP,
                sync_info=mybir.SyncInfo(on_wait=[w], on_update=[]),
                bass_nofuse=True,
            )
            self.nc.sync.add_instruction(nop)

    self.nc.all_engine_barrier()
    assert self.sems is not None
    popped = self.nc._tile_sem_poison_stack.pop()
    assert popped is self._sem_poison
    self.nc.clear_and_free_semaphores(list(self.sems.allocated().values()))
    self.nc.all_engine_barrier()


def _build_nc():
    import types

    import concourse.bass as bass
    import concourse.mybir as mybir
    from concourse.tile import TileContext

    dt = mybir.dt
    AF = mybir.ActivationFunctionType
    nc = bass.Bass()

    xin = nc.dram_tensor(
        "xin", [NPAIR, NCH * FB * P], dt.float8e4, kind="ExternalInput"
    )
    win = nc.dram_tensor("win", [P, KC * OUT], dt.float16, kind="ExternalInput")
    # cols 0:OUT: bias; cols OUT:OUT+BPC: ones (same row -- matmul
    # operands must start at partition 0/32/64)
    bin_ = nc.dram_tensor("bin", [1, OUT + BPC], dt.float32, kind="ExternalInput")
    yout = nc.dram_tensor("yout", [BPC, OUT], dt.float32, kind="ExternalOutput")

    # Walrus single-sync-wait discipline (see _split_drain_and_barrier):
    #  - x tiles get one pool slot per DMA (no slot reuse -> DMAs need 0
    #    waits), per-pair psum G tiles are not reused
    #  - the s columns are read/reshaped on DVE only; cross-engine joins
    #    funnel through DVE ticks so same-engine waits merge
    #  - PE "observes" each W slice's DMA lane via a dummy matmul right
    #    before the first fc matmul that reads the slice.
    tc = TileContext(nc)
    tc._drain_and_barrier = types.MethodType(_split_drain_and_barrier, tc)
    with tc:
        with (
            tc.tile_pool(name="const", bufs=1) as cpool,
            tc.tile_pool(name="xp", bufs=len(DMA_TILES) * NPAIR) as xpool,
            tc.tile_pool(name="small", bufs=2) as spool,
            tc.tile_pool(name="featp", bufs=1) as fpool,
            tc.tile_pool(name="gpsum", bufs=NPAIR, space="PSUM") as gpool,
            tc.tile_pool(name="rpsum", bufs=1, space="PSUM") as rpool,
            tc.tile_pool(name="opsum", bufs=1, space="PSUM") as opool,
        ):
            w_sb = cpool.tile([P, KC * OUT], dt.float16)
            bias_sb = cpool.tile([1, OUT + BPC], dt.float32)
            nc.scalar.dma_start(out=bias_sb[:], in_=bin_[:])

            # The two HWDGE rings drain strictly in issue order per ring.
            ring = [nc.sync, nc.scalar]
            rr = [0]

            def ring_dma(out, in_, force=None):
                r = force if force is not None else rr[0] % 2
                if force is None:
                    rr[0] += 1
                ring[r].dma_start(out=out, in_=in_)

            # feat_sb[p, c, bb] = flattened cov for sample bb, fc-chunk
            # layout: element k = c*128 + p of cov.flatten(); chunk c stacks
            # cov[:, 2c] on partitions 0:64 and cov[:, 2c+1] on 64:128.
            feat_sb = fpool.tile([P, KC, BPC], dt.float16)

            po = opool.tile([BPC, OUT], dt.float32)
            pdum = opool.tile([1, 512], dt.float32, tag="pdum")

            # One psum bank holds all four samples' rps regions at disjoint
            # column offsets; each region is only touched by its own chain,
            # so interleaved start=True zero-marking never clobbers live
            # data.
            rq = rpool.tile([D, BPC * D], dt.float32, tag="rq")

            # Scratch for the s-column -> s-row transpose. Only column 0 is
            # ever written; the 32x32 block transpose routes in-column j to
            # out-row j, so the junk in columns 1:32 lands only on output
            # rows we never read (everything but rows 0 and 32).
            s32 = cpool.tile([D, 32], dt.float32, tag="s32")
            # rsb[64q:64q+64] holds R = s s^T/(N(N-1)) for pair-member q;
            # the base partition matches ge's so the fused feat ops see all
            # inputs at one partition offset.
            rsb = cpool.tile([P, D], dt.float32, tag="rsb")

            # Pre-warm the PE clock gate (HAM) with dummy matmuls on a memset
            # tile while the first x tile is still in flight: the gate needs
            # ~3.4 us of sustained activity to lift the 1.2 GHz cold throttle.
            dumsrc = cpool.tile([P, 512], dt.float8e4)
            nc.vector.memset(dumsrc[:], 0.5)
            for _ in range(8):
                nc.tensor.matmul(
                    pdum[:], lhsT=dumsrc[:, 0:1], rhs=dumsrc[:, 0:512],
                    start=True, stop=True,
                )

            def do_pair(q):
                # pg[0:64, 0:64] = G of sample 2q, pg[64:128, 64:128] = G of
                # sample 2q+1, pg[64q':64q'+64, 128] = s of each. The
                # off-diagonal blocks are cross-sample junk (finite, unread).
                pg = gpool.tile([P, FB], dt.float32, tag="pg")
                xts = []
                # Ring0's sequencer reaches its first dma_start ~4 us before
                # ring1's (ring1 is behind the bias DMA and ACT preamble), so
                # pair 0's leading tiles all go to ring0 and ring1 carries
                # the tails -- this keeps tile arrival ahead of the PE's
                # ~2 us/tile consumption with no mid-stream stalls.
                RINGS = ([0, 0, 0, 0, 1, 1], [0, 0, 1, 1, 1, 1])[q]
                for ti, (i0, nblk) in enumerate(DMA_TILES):
                    xt = xpool.tile([P, nblk * FB], dt.float8e4, tag="xt")
                    xts.append(xt)
                    ring_dma(
                        xt[:],
                        xin[q, i0 * FB * P : (i0 + nblk) * FB * P].rearrange(
                            "(p f) -> p f", p=P
                        ),
                        force=RINGS[ti],
                    )
                for ti, (i0, nblk) in enumerate(DMA_TILES):
                    xt = xts[ti]
                    for j in range(nblk):
                        # 128-column stationary (both samples' x) triggers
                        # Fast Weight Load; the moving operand adds the ones
                        # column so column sums accumulate in psum col 128.
                        nc.tensor.matmul(
                            pg[:],
                            lhsT=xt[:, j * FB : j * FB + P],
                            rhs=xt[:, j * FB : (j + 1) * FB],
                            start=(i0 + j == 0),
                            stop=(i0 + j == NCH - 1),
                        )
                    # HAM-warming filler: keeps the PE activity monitor from
                    # re-throttling the clock during DMA stalls.
                    for _ in range(FILL_PER_TILE):
                        nc.tensor.matmul(
                            pdum[:, 0:256], lhsT=xt[:, 0:1], rhs=xt[:, 0:256],
                            start=True, stop=True,
                        )
                for qq in range(2):
                    bb = 2 * q + qq
                    base = D * qq
                    # s column -> row: copy into col 0 of s32, 32x32 block
                    # transpose, stitch the two 32-halves (rows 0 and 32).
                    nc.vector.tensor_copy(
                        s32[:, 0:1], pg[base : base + D, 2 * D : FB]
                    )
                    sT = spool.tile([D, 32], dt.float32, tag="sT")
                    nc.vector.transpose(sT[:], s32[:])
                    s_pos = spool.tile([1, D], dt.float32, tag="spos")
                    nc.vector.tensor_copy(s_pos[0:1, 0:32], sT[0:1, 0:32])
                    nc.vector.tensor_copy(s_pos[0:1, 32:D], sT[32:33, 0:32])
                    s_scl = spool.tile([1, D], dt.float32, tag="sscl")
                    nc.vector.tensor_scalar_mul(
                        s_scl[:], s_pos[:], 1.0 / (N * (N - 1.0))
                    )
                    rps = rq[:, bb * D : (bb + 1) * D]
                    nc.tensor.matmul(
                        rps, lhsT=s_scl[:], rhs=s_pos[:], start=True,
                        stop=True, skip_group_check=True,
                    )
                    nc.vector.tensor_copy(rsb[base : base + D, :], rps)
                    ge = pg[base : base + D, base : base + D].rearrange(
                        "p (c two) -> p c two", two=2
                    )
                    re = rsb[base : base + D, :].rearrange(
                        "p (c two) -> p c two", two=2
                    )
                    # feat = G/(N-1) - s s^T/(N(N-1))  (= cov), cast to fp16
                    nc.vector.scalar_tensor_tensor(
                        feat_sb[0:D, :, bb], ge[:, :, 0], 1.0 / (N - 1.0),
                        re[:, :, 0], op0=mybir.AluOpType.mult,
                        op1=mybir.AluOpType.subtract,
                    )
                    nc.vector.scalar_tensor_tensor(
                        feat_sb[D:P, :, bb], ge[:, :, 1], 1.0 / (N - 1.0),
                        re[:, :, 1], op0=mybir.AluOpType.mult,
                        op1=mybir.AluOpType.subtract,
                    )
                # keep the PE array warm across the pair-boundary stall
                for _ in range(0 if q == 0 else 2):
                    nc.tensor.matmul(
                        pdum[:, 0:256], lhsT=xts[-1][:, 0:1],
                        rhs=xts[-1][:, 0:256], start=True, stop=True,
                    )

            for q in range(NPAIR):
                do_pair(q)

            # W rides both rings AFTER the entire x stream: the fc matmuls
            # pace behind the arriving W slices and hide under their DMA
            # time instead of trailing the x stream.
            WSL = KC * OUT // WSLICES
            WRINGS = [0, 1, 0, 1, 0, 1, 1, 1]  # rebalance bytes across rings
            for c in range(WSLICES):
                ring_dma(
                    w_sb[:, c * WSL : (c + 1) * WSL],
                    win[:, c * WSL : (c + 1) * WSL],
                    force=WRINGS[c],
                )

            # Bridge the PE idle gap while the last pair's feat chain runs on
            # DVE: without activity the HAM re-throttles the clock to 1.2 GHz
            # and the fc then runs at half speed.
            for _ in range(10):
                nc.tensor.matmul(
                    pdum[:, 0:256], lhsT=dumsrc[:, 0:1], rhs=dumsrc[:, 0:256],
                    start=True, stop=True,
                )
            # Open the fc accumulation with the bias row: po = 1 * bias'.
            nc.tensor.matmul(
                po[:], lhsT=bias_sb[0:1, OUT : OUT + BPC], rhs=bias_sb[0:1, 0:OUT],
                start=True, stop=False,
            )
            # fc: out[bb, o] = bias'[o] + sum_k feat[k, bb] * W[k, o].
            # Before the first chunk of each W slice, a 1x1 dummy matmul
            # observes that slice's DMA lane so the fc matmul itself only
            # needs its feat (DVE) wait.
            CPS = KC // WSLICES
            for c in range(KC):
                if c % CPS == 0:
                    sl = c // CPS
                    nc.tensor.matmul(
                        pdum[0:1, 0:1],
                        lhsT=w_sb[0:1, sl * WSL : sl * WSL + 1],
                        rhs=w_sb[0:1, sl * WSL : sl * WSL + 1],
                        start=True, stop=True,
                    )
                nc.tensor.matmul(
                    po[:],
                    lhsT=feat_sb[:, c, :],
                    rhs=w_sb[:, c * OUT : (c + 1) * OUT],
                    start=False,
                    stop=(c == KC - 1),
                )

            # L2 normalize rows: out = po / sqrt(sum(po^2)). ACT Square with
            # row-sum accumulator (square and sqrt share one ACT table set),
            # ACT sqrt, DVE reciprocal, one DVE scale.
            sq = spool.tile([BPC, OUT], dt.float32, tag="sq")
            ss = spool.tile([BPC, 1], dt.float32, tag="ss")
            nc.scalar.activation(sq[:], po[:], AF.Square, accum_out=ss[:])
            nrm = spool.tile([BPC, 1], dt.float32, tag="nrm")
            nc.scalar.activation(nrm[:], ss[:], AF.Sqrt)
            inv = spool.tile([BPC, 1], dt.float32, tag="inv")
            nc.vector.reciprocal(inv[:], nrm[:])
            out_sb = spool.tile([BPC, OUT], dt.float32, tag="osb")
            nc.vector.tensor_scalar_mul(out_sb[:], po[:], inv[:])
            # SWDGE: an HWDGE yout DMA would need a DMAHW lane-reuse wait on
            # top of its DVE data wait (2 waits > walrus limit).
            nc.gpsimd.dma_start(out=yout[:], in_=out_sb[:])

    return nc


def _get_nc():
    if "nc" not in _CACHE:
        _CACHE["nc"] = _build_nc()
    return _CACHE["nc"]


def _pack_inputs(x, W, b):
    x = np.asarray(x, dtype=np.float32)
    W = np.asarray(W, dtype=np.float32)
    b = np.asarray(b, dtype=np.float32)

    xpad = np.zeros((B, NPAD, D), dtype=ml_dtypes.float8_e4m3)
    xpad[:, :N, :] = x.astype(ml_dtypes.float8_e4m3)
    # Pair samples (2q, 2q+1); chunk i, partition p holds row i*128+p of
    # both samples plus a shared ones byte: [x_a(64) | x_b(64) | 1].
    # [B,NPAD,D] -> [B/2, 2, NCH, P, D] -> [B/2, P, NCH, 2, D]
    xq = xpad.reshape(B // 2, 2, NCH, P, D).transpose(0, 3, 2, 1, 4)
    xq = xq.reshape(B // 2, P, NCH, 2 * D)
    ones = np.ones((B // 2, P, NCH, 1), dtype=ml_dtypes.float8_e4m3)
    augT = np.concatenate([xq, ones], axis=3).reshape(B // 2, P, NCH * FB)
    # regroup into DMA tiles: each dma_start reads one contiguous extent
    parts = []
    for (i0, nblk) in DMA_TILES:
        blk = augT[:, :, i0 * FB : (i0 + nblk) * FB]
        parts.append(blk.reshape(B // 2, P * nblk * FB))
    augT = np.ascontiguousarray(np.concatenate(parts, axis=1))

    wp = np.ascontiguousarray(
        W.reshape(KC, P, OUT).transpose(1, 0, 2)
    ).reshape(P, KC * OUT).astype(np.float16)
    bp = np.concatenate([b, np.ones(BPC, np.float32)]).reshape(1, OUT + BPC)

    return [
        {
            "xin": np.ascontiguousarray(augT[c * NPAIR : (c + 1) * NPAIR]),
            "win": wp,
            "bin": bp,
        }
        for c in range(NCORES)
    ]


def run(x, W, b, trace=False):
    from concourse.bass_utils import run_bass_kernel_spmd

    nc = _get_nc()
    in_maps = _pack_inputs(x, W, b)
    res = run_bass_kernel_spmd(nc, in_maps, list(range(NCORES)), trace=trace)
    out = np.concatenate(
        [res.results[c]["yout"] for c in range(NCORES)], axis=0
    ).astype(np.float32)
    return out, res


def kernel(x, W, b):
    out, _ = run(x, W, b, trace=False)
    return out



# revision 8
# speedup vs baseline: 1.1117x; 1.1117x over previous
"""Trainium2 Bass kernel for per-sample covariance pooling + fc + L2 norm.

Reference computation (per sample b of B=32):
    xc  = x[b] - mean(x[b], axis=0)            # x[b]: [N=20000, D=64]
    cov = xc.T @ xc / (N-1)                    # [64, 64]
    out = normalize(cov.flatten() @ W + b)     # [256]

Kernel formulation (scale/norm invariant):
    G = x.T @ x, s = sum(x, axis=0)            # one PE pass over x
    cov = (G - s s^T / N) / (N-1)
    out = normalize(cov.flatten() @ W + b)

Sharding: data-parallel over batch, 4 samples per core on 8 cores; W
and bias replicated. x is host-packed to fp8 e4m3 (end-to-end rel err
~2.3e-3 vs the 2e-2 gate). Two samples ride side by side per
partition row: chunk layout [x_a(64) | x_b(64) | ones(1)], so the
Gram matmul has a 128-column stationary operand (Fast Weight Load)
and one matmul per 128 rows yields both samples' G blocks plus both
column sums (from the ones column) in a [128, 129] psum.

The rank-1 mean correction is folded into the same psum accumulation:
after the Gram stream, s is transposed to a row (32x32 DVE block
transpose), scaled by -1/N, and eight tiny outer-product matmuls
accumulate -s s^T/N into the two G blocks (32x32 sub-blocks at
partition bases 0/32/64/96). feat = pg * 1/(N-1) then needs only a
plain tensor_scalar per parity half -- no separate R psum, no
SBUF-staged R, no stitch copies.

DMA schedule: pair-0 starts with small tiles (8/16/20 chunks) so the
first Gram matmul can start ~2 us earlier; tiles alternate between
the two HWDGE rings in consumption order (each ring sustains only
~170 GB/s; together ~340). The bias rides ring1 *after* the x stream
(it previously delayed ring1's first x tile by ~1.5 us) and W slices
alternate rings after all x. Warmup dummy matmuls lift the HAM clock
throttle before the first tile lands; a few bridge dummies keep the
clock up across the post-Gram DVE window.
"""

import sys

import numpy as np
import ml_dtypes

for _p in ("/opt/trn_rl_repo",):
    if _p not in sys.path:
        sys.path.append(_p)

# Problem shapes (hardcoded per contract).
B, N, D, OUT = 32, 20000, 64, 256
NCORES = 8
BPC = B // NCORES            # samples per core
NPAIR = BPC // 2             # sample pairs per core
P = 128                      # SBUF partitions / matmul contraction tile
NCH = (N + P - 1) // P       # 157 contraction chunks of 128 rows
NPAD = NCH * P               # 20096 rows after zero padding
FB = 2 * D + 1               # bytes per partition per chunk (pair + ones)
KC = (D * D) // P            # 32 fc contraction chunks
WSLICES = 8                  # W DMA slices (each covers 4 fc chunks)
# x DMA schedule per pair: (chunk offset, chunks). Pair 0 leads with
# small tiles so the Gram stream starts as soon as possible; pair 1
# has slack and uses big tiles.
DMA_TILES_P0 = [(0, 8), (8, 16), (24, 20), (44, 28), (72, 28), (100, 28), (128, 29)]
DMA_TILES_P1 = [(0, 28), (28, 28), (56, 28), (84, 28), (112, 28), (140, 17)]
TILES = [DMA_TILES_P0, DMA_TILES_P1]
# ring per tile, in consumption order (alternate so delivery uses both
# rings' bandwidth for the pair currently being consumed)
RINGS_P0 = [0, 1, 0, 1, 0, 1, 0]
RINGS_P1 = [1, 0, 1, 0, 1, 0]
TRINGS = [RINGS_P0, RINGS_P1]
WRINGS = [1, 0, 1, 0, 1, 0, 1, 0]

_CACHE = {}


def _split_drain_and_barrier(self, tick_clock, wait_clock):
    """Replacement for TileContext._drain_and_barrier emitting one drain per
    sem wait: this walrus vintage rejects >1 sync-wait per instruction."""
    import bass_rust
    import concourse.mybir as mybir

    drain_bi = self.nc.sync.drain()
    inst = drain_bi.ins
    wait_clock.add_sem_waits(
        drain_bi.ins, bass_rust.ScopedClock({None: tick_clock.global_clock})
    )
    waits = list(inst.sync_info.on_wait) if inst.sync_info else []
    if len(waits) > 1:
        # one pure sem-wait NoOp per extra wait (cheaper than extra drains)
        inst.sync_info = mybir.SyncInfo(on_wait=waits[:1], on_update=[])
        for w in waits[1:]:
            nop = mybir.InstNoOp(
                name=f"tailwait-{w.ant_name}",
                engine=mybir.EngineType.SP,
                sync_info=mybir.SyncInfo(on_wait=[w], on_update=[]),
                bass_nofuse=True,
            )
            self.nc.sync.add_instruction(nop)

    self.nc.all_engine_barrier()
    assert self.sems is not None
    popped = self.nc._tile_sem_poison_stack.pop()
    assert popped is self._sem_poison
    self.nc.clear_and_free_semaphores(list(self.sems.allocated().values()))
    self.nc.all_engine_barrier()


def _build_nc():
    import types

    import concourse.bass as bass
    import concourse.mybir as mybir
    from concourse.tile import TileContext

    dt = mybir.dt
    nc = bass.Bass()

    xin = nc.dram_tensor(
        "xin", [NPAIR, NCH * FB * P], dt.float8e4, kind="ExternalInput"
    )
    win = nc.dram_tensor("win", [P, KC * OUT], dt.float16, kind="ExternalInput")
    # cols 0:OUT: bias; cols OUT:OUT+BPC: ones (same row -- matmul
    # operands must start at a 32-multiple partition)
    bin_ = nc.dram_tensor("bin", [1, OUT + BPC], dt.float32, kind="ExternalInput")
    yout = nc.dram_tensor("yout", [BPC, OUT], dt.float32, kind="ExternalOutput")

    # Walrus single-sync-wait discipline (see _split_drain_and_barrier):
    #  - x tiles get one pool slot per DMA (no slot reuse -> DMAs need 0
    #    waits), per-pair psum G tiles are not reused
    #  - cross-engine joins funnel through single producers so each
    #    consumer carries at most one sem wait
    #  - PE "observes" each W slice's DMA lane via a dummy matmul right
    #    before the first fc matmul that reads the slice.
    tc = TileContext(nc)
    tc._drain_and_barrier = types.MethodType(_split_drain_and_barrier, tc)
    with tc:
        with (
            tc.tile_pool(name="const", bufs=1) as cpool,
            tc.tile_pool(name="xp", bufs=len(DMA_TILES_P0) + len(DMA_TILES_P1)) as xpool,
            tc.tile_pool(name="small", bufs=4) as spool,
            tc.tile_pool(name="featp", bufs=1) as fpool,
            tc.tile_pool(name="gpsum", bufs=NPAIR, space="PSUM") as gpool,
            tc.tile_pool(name="opsum", bufs=1, space="PSUM") as opool,
        ):
            w_sb = cpool.tile([P, KC * OUT], dt.float16)
            bias_sb = cpool.tile([1, OUT + BPC], dt.float32)

            ring = [nc.sync, nc.scalar]

            # feat_sb[p, c, bb] = flattened cov for sample bb, fc-chunk
            # layout: element k = c*128 + p of cov.flatten(); chunk c stacks
            # cov[:, 2c] on partitions 0:64 and cov[:, 2c+1] on 64:128.
            feat_sb = fpool.tile([P, KC, BPC], dt.float16)

            po = opool.tile([BPC, OUT], dt.float32)
            pdum = opool.tile([1, 512], dt.float32, tag="pdum")

            # s column scratch (only col 0 written; the 32x32 block
            # transpose routes in-col j to out-row j, so the junk in cols
            # 1:32 lands only on output rows we never read).
            s128 = cpool.tile([P, 32], dt.float32, tag="s128")

            # Pre-warm the PE clock gate (HAM) with dummy matmuls on a memset
            # tile while the first x tile is still in flight: the gate needs
            # ~3.4 us of sustained activity to lift the cold throttle.
            dumsrc = cpool.tile([P, 512], dt.float8e4)
            nc.vector.memset(dumsrc[:], 0.5)

            def dummy_mm(n, cols=256):
                for _ in range(n):
                    nc.tensor.matmul(
                        pdum[:, 0:cols], lhsT=dumsrc[:, 0:1],
                        rhs=dumsrc[:, 0:cols], start=True, stop=True,
                    )

            # ---- all x DMAs up front, alternating rings in consumption
            # order; tile delivery is serial per ring (~170 GB/s each).
            xts = [[], []]
            for q in range(NPAIR):
                for ti, (i0, nblk) in enumerate(TILES[q]):
                    xt = xpool.tile([P, nblk * FB], dt.float8e4, tag="xt")
                    xts[q].append(xt)
                    ring[TRINGS[q][ti]].dma_start(
                        out=xt[:],
                        in_=xin[q, i0 * FB * P : (i0 + nblk) * FB * P].rearrange(
                            "(p f) -> p f", p=P
                        ),
                    )
            # bias + W ride the rings behind the whole x stream.
            nc.scalar.dma_start(out=bias_sb[:], in_=bin_[:])
            WSL = KC * OUT // WSLICES
            for c in range(WSLICES):
                ring[WRINGS[c]].dma_start(
                    out=w_sb[:, c * WSL : (c + 1) * WSL],
                    in_=win[:, c * WSL : (c + 1) * WSL],
                )

            dummy_mm(6, cols=512)

            pgs = []

            def gram(q):
                # pg[0:64, 0:64] = G of sample 2q, pg[64:128, 64:128] = G of
                # sample 2q+1, pg[64q', 128] = s of each. The off-diagonal
                # blocks are cross-sample junk (finite, unread).
                pg = gpool.tile([P, FB], dt.float32, tag="pg")
                pgs.append(pg)
                for ti, (i0, nblk) in enumerate(TILES[q]):
                    xt = xts[q][ti]
                    for j in range(nblk):
                        nc.tensor.matmul(
                            pg[:],
                            lhsT=xt[:, j * FB : j * FB + P],
                            rhs=xt[:, j * FB : (j + 1) * FB],
                            start=(i0 + j == 0),
                            stop=False,
                        )

            def schain(q):
                # s (psum col 128, both samples stacked) -> SBUF column ->
                # 32x32 block transpose puts s[32k:32k+32] into row 32k ->
                # stitch the four 32-wide pieces into a [1, 128] row (plain
                # and scaled by -1/N) so the outer-product matmul sees both
                # operands at partition 0.
                pg = pgs[q]
                nc.vector.tensor_copy(s128[:, 0:1], pg[:, 2 * D : FB])
                sT = spool.tile([P, 32], dt.float32, tag="sT")
                nc.vector.transpose(sT[:], s128[:])
                spos = spool.tile([1, P], dt.float32, tag="spos")
                sscl = spool.tile([1, P], dt.float32, tag="sscl")
                for h in range(4):
                    nc.vector.tensor_copy(
                        spos[0:1, 32 * h : 32 * h + 32], sT[32 * h : 32 * h + 1, :]
                    )
                    nc.vector.tensor_scalar_mul(
                        sscl[0:1, 32 * h : 32 * h + 32],
                        sT[32 * h : 32 * h + 1, :], -1.0 / N,
                    )
                return spos, sscl

            def rank1(q, spos, sscl):
                # accumulate -s s^T / N into each sample's G block.
                pg = pgs[q]
                for bb in range(2):
                    base = D * bb
                    nc.tensor.matmul(
                        pg[base : base + D, base : base + D],
                        lhsT=sscl[0:1, base : base + D],
                        rhs=spos[0:1, base : base + D],
                        start=False,
                        stop=(bb == 1),
                        skip_group_check=True,
                    )

            def feats(q, halves):
                # feat = pg * 1/(N-1)  (= cov), cast to fp16. Two parity
                # halves per sample; optionally split the chunk range so the
                # fc can start while the second half is still on DVE.
                pg = pgs[q]
                HC = KC // halves
                for h in range(halves):
                    for bb in range(2):
                        base = D * bb
                        ge = pg[base : base + D, base : base + D].rearrange(
                            "p (c two) -> p c two", two=2
                        )
                        for par in range(2):
                            nc.vector.tensor_scalar_mul(
                                feat_sb[par * D : par * D + D,
                                        h * HC : (h + 1) * HC, 2 * q + bb],
                                ge[:, h * HC : (h + 1) * HC, par],
                                1.0 / (N - 1.0),
                            )

            # pair 0 Gram stream, then its s-chain on DVE while pair 1's
            # stream begins; pair 0's rank-1 matmuls slot in after pair 1's
            # first tile so they never stall the PE (their sT input is long
            # ready by then).
            gram(0)
            spos0, sscl0 = schain(0)

            # pair 1 gram: first tile, then pair-0 rank1 + feats, then rest
            pg1 = gpool.tile([P, FB], dt.float32, tag="pg")
            pgs.append(pg1)
            i0, nblk = TILES[1][0]
            xt = xts[1][0]
            for j in range(nblk):
                nc.tensor.matmul(
                    pg1[:], lhsT=xt[:, j * FB : j * FB + P],
                    rhs=xt[:, j * FB : (j + 1) * FB],
                    start=(i0 + j == 0), stop=False,
                )
            rank1(0, spos0, sscl0)
            feats(0, halves=1)
            for ti in range(1, len(TILES[1])):
                i0, nblk = TILES[1][ti]
                xt = xts[1][ti]
                for j in range(nblk):
                    nc.tensor.matmul(
                        pg1[:], lhsT=xt[:, j * FB : j * FB + P],
                        rhs=xt[:, j * FB : (j + 1) * FB],
                        start=False, stop=False,
                    )
            # fix pgs bookkeeping: pgs[1] is pg1
            spos1, sscl1 = schain(1)
            dummy_mm(4)          # keep PE active while DVE runs the s-chain
            rank1(1, spos1, sscl1)
            dummy_mm(6)          # bridge the feat window (HAM re-throttle)
            feats(1, halves=2)

            # Open the fc accumulation with the bias row: po = 1 * bias'.
            nc.tensor.matmul(
                po[:], lhsT=bias_sb[0:1, OUT : OUT + BPC], rhs=bias_sb[0:1, 0:OUT],
                start=True, stop=False,
            )
            # fc: out[bb, o] = bias'[o] + sum_k feat[k, bb] * W[k, o].
            # Before the first chunk of each W slice, a 1x1 dummy matmul
            # observes that slice's DMA lane so the fc matmul itself only
            # needs its feat (DVE) wait.
            CPS = KC // WSLICES
            for c in range(KC):
                if c % CPS == 0:
                    sl = c // CPS
                    nc.tensor.matmul(
                        pdum[0:1, 0:1],
                        lhsT=w_sb[0:1, sl * WSL : sl * WSL + 1],
                        rhs=w_sb[0:1, sl * WSL : sl * WSL + 1],
                        start=True, stop=True,
                    )
                nc.tensor.matmul(
                    po[:],
                    lhsT=feat_sb[:, c, :],
                    rhs=w_sb[:, c * OUT : (c + 1) * OUT],
                    start=False,
                    stop=(c == KC - 1),
                )

            # L2 normalize rows: out = po / sqrt(sum(po^2)). ACT Square with
            # row-sum accumulator (a DVE square would need two PSUM reads),
            # ACT sqrt, DVE reciprocal, one DVE scale.
            sq = spool.tile([BPC, OUT], dt.float32, tag="sq")
            ss = spool.tile([BPC, 1], dt.float32, tag="ss")
            nc.scalar.activation(
                sq[:], po[:], mybir.ActivationFunctionType.Square, accum_out=ss[:]
            )
            nrm = spool.tile([BPC, 1], dt.float32, tag="nrm")
            nc.scalar.activation(nrm[:], ss[:], mybir.ActivationFunctionType.Sqrt)
            inv = spool.tile([BPC, 1], dt.float32, tag="inv")
            nc.vector.reciprocal(inv[:], nrm[:])
            out_sb = spool.tile([BPC, OUT], dt.float32, tag="osb")
            nc.vector.tensor_scalar_mul(out_sb[:], po[:], inv[:])
            # SWDGE: an HWDGE yout DMA would need a DMAHW lane-reuse wait on
            # top of its DVE data wait (2 waits > walrus limit).
            nc.gpsimd.dma_start(out=yout[:], in_=out_sb[:])

    return nc


def _get_nc():
    if "nc" not in _CACHE:
        _CACHE["nc"] = _build_nc()
    return _CACHE["nc"]


def _pack_inputs(x, W, b):
    x = np.asarray(x, dtype=np.float32)
    W = np.asarray(W, dtype=np.float32)
    b = np.asarray(b, dtype=np.float32)

    xpad = np.zeros((B, NPAD, D), dtype=ml_dtypes.float8_e4m3)
    xpad[:, :N, :] = x.astype(ml_dtypes.float8_e4m3)
    # Pair samples (2q, 2q+1); chunk i, partition p holds row i*128+p of
    # both samples plus a shared ones byte: [x_a(64) | x_b(64) | 1].
    # [B,NPAD,D] -> [B/2, 2, NCH, P, D] -> [B/2, P, NCH, 2, D]
    xq = xpad.reshape(B // 2, 2, NCH, P, D).transpose(0, 3, 2, 1, 4)
    xq = xq.reshape(B // 2, P, NCH, 2 * D)
    ones = np.ones((B // 2, P, NCH, 1), dtype=ml_dtypes.float8_e4m3)
    augT = np.concatenate([xq, ones], axis=3).reshape(B // 2, P, NCH * FB)
    # regroup into DMA tiles: each dma_start reads one contiguous extent.
    # Pair 0 and pair 1 of each core use different tile schedules.
    rows = []
    for gp in range(B // 2):
        tiles = TILES[gp % NPAIR]
        parts = [
            np.ascontiguousarray(augT[gp, :, i0 * FB : (i0 + nblk) * FB]).reshape(-1)
            for (i0, nblk) in tiles
        ]
        rows.append(np.concatenate(parts))
    augT = np.stack(rows)

    wp = np.ascontiguousarray(
        W.reshape(KC, P, OUT).transpose(1, 0, 2)
    ).reshape(P, KC * OUT).astype(np.float16)
    bp = np.concatenate([b, np.ones(BPC, np.float32)]).reshape(1, OUT + BPC)

    return [
        {
            "xin": np.ascontiguousarray(augT[c * NPAIR : (c + 1) * NPAIR]),
            "win": wp,
            "bin": bp,
        }
        for c in range(NCORES)
    ]


def run(x, W, b, trace=False):
    from concourse.bass_utils import run_bass_kernel_spmd

    nc = _get_nc()
    in_maps = _pack_inputs(x, W, b)
    res = run_bass_kernel_spmd(nc, in_maps, list(range(NCORES)), trace=trace)
    out = np.concatenate(
        [res.results[c]["yout"] for c in range(NCORES)], axis=0
    ).astype(np.float32)
    return out, res


def kernel(x, W, b):
    out, _ = run(x, W, b, trace=False)
    return out
